# revision 24
# baseline (speedup 1.0000x reference)
"""Trainium2 Bass kernel for a dense transformer block.

Block: split-LayerNorm -> attention -> residual -> split-LayerNorm -> MLP(GELU)
-> residual.  Shapes: B=8, N=1024, D=768, H=12 heads (hd=64), HID=3072.

Sharding: pure data-parallel over batch -- one batch element per NeuronCore
(8 cores), all weights replicated, no collectives.

On-chip layout is feature-major (activations stored transposed, [feature, token])
so every matmul consumes activations as the moving operand directly and
LayerNorm/softmax cross-feature reductions map onto PE matmuls with
indicator/ones vectors.  The host pre-transposes x and all weight matrices so
every DMA is contiguous.
"""

import os
import numpy as np
import ml_dtypes

import concourse.bass as bass
import concourse.tile as tile
from concourse import bacc, mybir
from concourse.bass_utils import run_bass_kernel_spmd
from concourse.masks import make_identity
from contextlib import ExitStack

F32 = mybir.dt.float32
BF16 = mybir.dt.bfloat16
F32R = mybir.dt.float32r
AF = mybir.ActivationFunctionType
ALU = mybir.AluOpType

D = 768
H = 12
HD = 64
HID = 3072
NT = 1024  # tokens per core
B = 8
S1 = 320  # split-LN segment boundaries: [0,320), [320,384), [384,768)
S2 = 384
SCALE = 0.125  # (D//H) ** -0.5 = 64 ** -0.5
EPS = 1e-5
P = 128

KC = D // P        # 6  c-chunks
MQKV = 3 * D // P  # 18 qkv out chunks
MFC1 = HID // P    # 24


def _halves():
    return (slice(0, 512), slice(512, 1024))


# Per segment: (ind_bT column range for the broadcast lhsT,
#               [(chunk, row0, row1), ...] applied regions)
LN_REGIONS = [
    (slice(0, P), [(0, 0, P), (1, 0, P), (2, 0, HD)]),        # seg0 [0,320)
    (slice(S1, S2), [(2, HD, P)]),                            # seg1 [320,384)
    (slice(S2, S2 + P), [(3, 0, P), (4, 0, P), (5, 0, P)]),   # seg2 [384,768)
]


def _layernorm(tc, x_tiles, g_tile, b_tile, ind_sum_tiles, ind_bT, invlen,
               eps_t, psmm, out_pool, tag):
    """Split-LayerNorm over the feature dim (partitions).  x_tiles: 6 fp32
    [128,1024] feature-major tiles.  Returns 6 bf16 normalized tiles."""
    nc = tc.nc
    with tc.tile_pool(name=f"ln_{tag}", bufs=3) as lnp, \
         tc.tile_pool(name=f"lns_{tag}", bufs=1) as lns, \
         tc.tile_pool(name=f"lnps_{tag}", bufs=2, space="PSUM") as psstat:
        # segment sums via indicator matmuls: sums[s, q] = sum_{c in seg_s} x[c, q]
        sums_t = psstat.tile([P, NT], F32, tag="stat")
        sumsq_t = psstat.tile([P, NT], F32, tag="stat")
        sums = sums_t[0:3, :]
        sumsq = sumsq_t[0:3, :]
        for k in range(KC):
            xbk = lnp.tile([P, NT], BF16, tag="xb")
            nc.vector.tensor_copy(xbk[:], x_tiles[k][:])
            xqk = lnp.tile([P, NT], BF16, tag="xq")
            nc.vector.tensor_mul(xqk[:], x_tiles[k][:], x_tiles[k][:])
            for hs in _halves():
                nc.tensor.matmul(sums_t[:, hs], ind_sum_tiles[k][:],
                                 xbk[:, hs],
                                 start=(k == 0), stop=(k == KC - 1))
            for hs in _halves():
                nc.tensor.matmul(sumsq_t[:, hs], ind_sum_tiles[k][:],
                                 xqk[:, hs],
                                 start=(k == 0), stop=(k == KC - 1))
        # stats on [3, NT]
        mean = lns.tile([3, NT], F32, tag="mean")
        nc.vector.tensor_scalar_mul(mean[:], sums[:], invlen[:])
        nmsq = lns.tile([3, NT], F32, tag="nmsq")
        nc.vector.scalar_tensor_tensor(nmsq[:], mean[:], -1.0, mean[:],
                                       ALU.mult, ALU.mult)
        var = lns.tile([3, NT], F32, tag="var")
        nc.vector.scalar_tensor_tensor(var[:], sumsq[:], invlen[:], nmsq[:],
                                       ALU.mult, ALU.add)
        std = lns.tile([3, NT], F32, tag="std")
        nc.scalar.activation(std[:], var[:], AF.Sqrt, bias=eps_t[:])
        rstd = lns.tile([3, NT], F32, tag="rstd")
        scr = lns.tile([3, NT], F32, tag="scr")
        nc.vector.reciprocal_approx_accurate(rstd[:], std[:], scr[:])
        beta = lns.tile([3, NT], F32, tag="beta")
        nc.vector.scalar_tensor_tensor(beta[:], mean[:], -1.0, rstd[:],
                                       ALU.mult, ALU.mult)
        rstd_r = lns.tile([3, NT], F32R, tag="rstd_r")
        nc.vector.tensor_copy(rstd_r[:], rstd[:])
        beta_r = lns.tile([3, NT], F32R, tag="beta_r")
        nc.vector.tensor_copy(beta_r[:], beta[:])
        # Broadcast per-token stats across partitions (one f32r matmul per
        # segment, reused by every chunk-region in that segment) + apply.
        t1s = {}
        t2s = {}
        done_rows = {}
        out_tiles = [None] * KC
        for cols, regions in LN_REGIONS:
            m_rows = max(r1 - r0 for _, r0, r1 in regions)
            aB = psmm.tile([P, NT], F32, tag="mm")
            bB = psmm.tile([P, NT], F32, tag="mm")
            for hs in _halves():
                nc.tensor.matmul(aB[0:m_rows, hs],
                                 ind_bT[:, cols][:, 0:m_rows],
                                 rstd_r[:, hs], start=True, stop=True)
                nc.tensor.matmul(bB[0:m_rows, hs],
                                 ind_bT[:, cols][:, 0:m_rows],
                                 beta_r[:, hs], start=True, stop=True)
            for k, r0, r1 in regions:
                if k not in t1s:
                    t1k = lnp.tile([P, NT], F32, tag="t1")
                    t2k = lnp.tile([P, NT], F32, tag="t2")
                    t1s[k] = t1k
                    t2s[k] = t2k
                    done_rows[k] = 0
                gcol = g_tile[r0:r1, k:k + 1]
                # t1 = (x * g) * rstd_bcast ; t2 = (-mean*rstd*g) + t1
                nc.vector.scalar_tensor_tensor(
                    t1s[k][r0:r1, :], x_tiles[k][r0:r1, :], gcol,
                    aB[0:r1 - r0, :], ALU.mult, ALU.mult)
                nc.vector.scalar_tensor_tensor(
                    t2s[k][r0:r1, :], bB[0:r1 - r0, :], gcol,
                    t1s[k][r0:r1, :], ALU.mult, ALU.add)
                done_rows[k] += r1 - r0
                if done_rows[k] == P:
                    ok = out_pool.tile([P, NT], BF16, tag=f"normx_{tag}")
                    nc.scalar.activation(ok[:], t2s[k][:], AF.Identity,
                                         bias=b_tile[:, k:k + 1])
                    out_tiles[k] = ok
        return out_tiles


DEBUG = bool(int(os.environ.get("KBG_DEBUG", "0")))


def build():
    nc = bacc.Bacc("TRN2", target_bir_lowering=False, debug=False)
    dbg = {}
    if DEBUG:
        dbg["normx"] = nc.dram_tensor("dbg_normx", [D, NT], BF16, kind="ExternalOutput")
        dbg["qkvT"] = nc.dram_tensor("dbg_qkvT", [3 * D, NT], BF16, kind="ExternalOutput")
        dbg["yt"] = nc.dram_tensor("dbg_yt", [D, NT], BF16, kind="ExternalOutput")
        dbg["x1"] = nc.dram_tensor("dbg_x1", [D, NT], F32, kind="ExternalOutput")
        dbg["normx2"] = nc.dram_tensor("dbg_normx2", [D, NT], BF16, kind="ExternalOutput")
        dbg["hT"] = nc.dram_tensor("dbg_hT", [HID, NT], BF16, kind="ExternalOutput")


    xT = nc.dram_tensor("xT", [D, NT], F32, kind="ExternalInput")
    wqkvT = nc.dram_tensor("wqkvT", [D, 3 * D], BF16, kind="ExternalInput")
    wprojT = nc.dram_tensor("wprojT", [D, D], BF16, kind="ExternalInput")
    wfc1T = nc.dram_tensor("wfc1T", [D, HID], BF16, kind="ExternalInput")
    wfc2T = nc.dram_tensor("wfc2T", [HID, D], BF16, kind="ExternalInput")
    pbias = nc.dram_tensor("pbias", [D], F32, kind="ExternalInput")
    fc1b = nc.dram_tensor("fc1b", [HID], F32, kind="ExternalInput")
    fc2b = nc.dram_tensor("fc2b", [D], F32, kind="ExternalInput")
    g1d = nc.dram_tensor("g1", [D], F32, kind="ExternalInput")
    b1d = nc.dram_tensor("b1", [D], F32, kind="ExternalInput")
    g2d = nc.dram_tensor("g2", [D], F32, kind="ExternalInput")
    b2d = nc.dram_tensor("b2", [D], F32, kind="ExternalInput")
    indsum = nc.dram_tensor("indsum", [D, P], BF16, kind="ExternalInput")
    indbTd = nc.dram_tensor("indbT", [3, D], F32R, kind="ExternalInput")
    invlend = nc.dram_tensor("invlen", [3, 1], F32, kind="ExternalInput")
    outT = nc.dram_tensor("outT", [D, NT], F32, kind="ExternalOutput")

    with tile.TileContext(nc) as tc, ExitStack() as ctx:
        const = ctx.enter_context(tc.tile_pool(name="const", bufs=1))
        psmm = ctx.enter_context(tc.tile_pool(name="psmm", bufs=2, space="PSUM"))

        # constants
        ident = const.tile([P, P], BF16)
        make_identity(nc, ident)
        eps_t = const.tile([3, 1], F32)
        nc.vector.memset(eps_t[:], EPS)
        ones64 = const.tile([1, HD], F32)
        nc.vector.memset(ones64[:], 1.0)

        def load_cols(dram, n):
            t = const.tile([P, n], F32, tag=f"c_{dram.name}")
            nc.sync.dma_start(t[:], dram.ap().rearrange("(a p) -> p a", p=P))
            return t

        pb = load_cols(pbias, KC)
        f1b = load_cols(fc1b, MFC1)
        f2b = load_cols(fc2b, KC)
        g1 = load_cols(g1d, KC)
        b1 = load_cols(b1d, KC)
        g2 = load_cols(g2d, KC)
        b2 = load_cols(b2d, KC)
        ind_sum_tiles = []
        for k in range(KC):
            t = const.tile([P, P], BF16, tag=f"inds{k}")
            nc.sync.dma_start(t[:], indsum[k * P:(k + 1) * P, :])
            ind_sum_tiles.append(t)
        ind_bT = const.tile([3, D], F32R)
        nc.sync.dma_start(ind_bT[:], indbTd[:])
        invlen = const.tile([3, 1], F32)
        nc.sync.dma_start(invlen[:], invlend[:])

        x1pool = ctx.enter_context(tc.tile_pool(name="x1pool", bufs=KC))

        stage1 = ctx.enter_context(ExitStack())  # x0, lives through proj
        x0pool = stage1.enter_context(tc.tile_pool(name="x0pool", bufs=KC))
        x_tiles = []
        for k in range(KC):
            t = x0pool.tile([P, NT], F32, tag="x0")
            for qr in range(4):
                nc.sync.dma_start(t[qr * 32:(qr + 1) * 32, :],
                                  xT[k * P + qr * 32:k * P + (qr + 1) * 32, :])
            x_tiles.append(t)

        # ---- LN1 ----
        nx_stage = ctx.enter_context(ExitStack())  # normx, lives through qkv
        nx_pool = nx_stage.enter_context(tc.tile_pool(name="nx_pool", bufs=KC))
        normx = _layernorm(tc, x_tiles, g1, b1, ind_sum_tiles, ind_bT,
                           invlen, eps_t, psmm, nx_pool, "ln1")
        if DEBUG:
            for k in range(KC):
                nc.sync.dma_start(dbg["normx"][k * P:(k + 1) * P, :], normx[k][:])

        # ---- fused QKV + attention, one head-pair at a time ----
        # Pair j computes qkv chunks {j, 6+j, 12+j}, builds the pair's V
        # tiles, then runs both heads.  Attention's ACT-heavy exp stream
        # overlaps the next pair's PE-heavy qkv matmuls.
        y_stage = ctx.enter_context(ExitStack())
        y_pool = y_stage.enter_context(
            tc.tile_pool(name="y_pool", bufs=KC, side="right"))
        qkv_stage = ctx.enter_context(ExitStack())
        q_pool = qkv_stage.enter_context(
            tc.tile_pool(name="q_pool", bufs=9, side="right"))
        yt = []
        for _yi in range(KC):
            yt_t = y_pool.tile([P, NT], BF16, tag="yt")
            yt.append(yt_t)
        with tc.tile_pool(name="wqkv", bufs=KC) as wq_pool, \
             tc.tile_pool(name="v_pool", bufs=4) as v_pool, \
             tc.tile_pool(name="e_pool", bufs=3) as e_pool, \
             tc.tile_pool(name="kp_pool", bufs=3) as kp_pool, \
             tc.tile_pool(name="psot", bufs=2, space="PSUM") as psot, \
             tc.tile_pool(name="sm_pool", bufs=2) as sm_pool:
            wq = []
            for k in range(KC):
                t = wq_pool.tile([P, 3 * D], BF16, tag="wqkv")
                nc.sync.dma_start(t[:], wqkvT[k * P:(k + 1) * P, :])
                wq.append(t)

            # Per-head normalization tail, deferred into the next head's
            # loop so the DVE reciprocal latency hides under its matmuls.
            def make_tail(h, ot):
                po = (h % 2) * HD

                def tail():
                    dnm = sm_pool.tile([1, NT], F32, tag="dnm")
                    nc.vector.tensor_copy(dnm[:], ot[64:65, :])
                    r = sm_pool.tile([1, NT], F32, tag="recip")
                    nc.vector.reciprocal_approx_fast(r[:], dnm[:])
                    rbs = sm_pool.tile([HD, NT], F32, tag="rbs")
                    nc.gpsimd.partition_broadcast(rbs[:], r[:])
                    nc.vector.tensor_mul(yt[h // 2][po:po + HD, :],
                                         ot[0:HD, :], rbs[:])
                return tail

            pending_tail = None
            for j in range(KC):
                qkvT_j = {}
                for m in (j, 6 + j, 12 + j):
                    ps = psmm.tile([P, NT], F32, tag="mm")
                    for hs in _halves():
                        for k in range(KC):
                            nc.tensor.matmul(ps[:, hs],
                                             wq[k][:, m * P:(m + 1) * P],
                                             normx[k][:, hs],
                                             start=(k == 0), stop=(k == KC - 1))
                    qt = q_pool.tile([P, NT], BF16, tag="qkv")
                    nc.scalar.activation(qt[:], ps[:], AF.Copy)
                    qkvT_j[m] = qt
                    if DEBUG:
                        nc.sync.dma_start(dbg["qkvT"][m * P:(m + 1) * P, :],
                                          qt[:])
                # V token-major (plus the all-ones columns already in the
                # memset giving the softmax denominator at output row 64)
                vts = []
                vsl = qkvT_j[12 + j]
                for hh in range(2):
                    po = hh * HD
                    vt = v_pool.tile([P, 8 * P], BF16, tag="vaug")
                    nc.vector.memset(vt[:], 1.0)
                    idn = ident[po:po + HD, po:po + HD]
                    for kc in range(8):
                        trp = psmm.tile([P, HD], BF16, tag="mm")
                        nc.tensor.transpose(
                            trp[:], vsl[po:po + HD, kc * P:(kc + 1) * P], idn)
                        nc.vector.tensor_copy(vt[:, kc * P:kc * P + HD],
                                              trp[:])
                    vts.append(vt)
                for hh in range(2):
                    h = 2 * j + hh
                    po = hh * HD
                    qsl = qkvT_j[j]
                    ksl = qkvT_j[6 + j]
                    # K=128 zero-padded k tile: rows [po:po+64] hold this
                    # head's k, the other 64 rows are zero so the full-height
                    # q rhs contributes nothing outside this head (and the
                    # weight load takes the fast NumWeights==128 path).
                    kp = kp_pool.tile([P, NT], BF16, tag="kp")
                    nc.vector.memset(kp[HD - po:P - po, :], 0.0)
                    nc.vector.tensor_copy(kp[po:po + HD, :],
                                          ksl[po:po + HD, :])
                    ot = psot.tile([P, NT], F32, tag="ot")
                    exs = []
                    for kc in range(8):
                        st = psmm.tile([P, NT], F32, tag="mm")
                        for hs in _halves():
                            nc.tensor.matmul(
                                st[:, hs],
                                kp[:, kc * P:(kc + 1) * P],
                                qsl[:, hs],
                                start=True, stop=True)
                        ex = e_pool.tile([P, NT], BF16, tag="expst")
                        nc.scalar.activation(ex[:], st[:], AF.Exp, scale=SCALE)
                        exs.append(ex)
                        if kc >= 1:
                            exp_prev = exs[kc - 1]
                            for hs in _halves():
                                nc.tensor.matmul(
                                    ot[:, hs],
                                    vts[hh][:, (kc - 1) * P:(kc - 1) * P + P],
                                    exp_prev[:, hs],
                                    start=(kc == 1), stop=False)
                        if kc == 3 and pending_tail is not None:
                            pending_tail()
                            pending_tail = None
                    for hs in _halves():
                        nc.tensor.matmul(ot[:, hs],
                                         vts[hh][:, 7 * P:7 * P + P],
                                         exs[7][:, hs],
                                         start=False, stop=True)
                    pending_tail = make_tail(h, ot)
            pending_tail()
            if DEBUG:
                for k in range(KC):
                    nc.sync.dma_start(dbg["yt"][k * P:(k + 1) * P, :], yt[k][:])
        nx_stage.close()
        qkv_stage.close()

        # ---- proj + residual ----
        x1 = []
        with tc.tile_pool(name="wp_pool", bufs=KC) as wp_pool, \
             tc.tile_pool(name="prj_pool", bufs=2) as prj_pool:
            wp = []
            for k in range(KC):
                t = wp_pool.tile([P, D], BF16, tag="wp")
                nc.sync.dma_start(t[:], wprojT[k * P:(k + 1) * P, :])
                wp.append(t)
            for m in range(KC):
                ps = psmm.tile([P, NT], F32, tag="mm")
                for hs in _halves():
                    for k in range(KC):
                        nc.tensor.matmul(ps[:, hs],
                                         wp[k][:, m * P:(m + 1) * P],
                                         yt[k][:, hs],
                                         start=(k == 0), stop=(k == KC - 1))
                tp = prj_pool.tile([P, NT], F32, tag="tp")
                nc.scalar.activation(tp[:], ps[:], AF.Identity,
                                     bias=pb[:, m:m + 1])
                xk = x1pool.tile([P, NT], F32, tag="x1")
                nc.vector.tensor_add(xk[:], x_tiles[m][:], tp[:])
                x1.append(xk)
                if DEBUG:
                    nc.sync.dma_start(dbg["x1"][m * P:(m + 1) * P, :], xk[:])
        y_stage.close()
        stage1.close()

        # ---- LN2 ----
        nx2_stage = ctx.enter_context(ExitStack())
        nx2_pool = nx2_stage.enter_context(tc.tile_pool(name="nx2_pool",
                                                        bufs=KC))
        normx2 = _layernorm(tc, x1, g2, b2, ind_sum_tiles, ind_bT,
                            invlen, eps_t, psmm, nx2_pool, "ln2")
        if DEBUG:
            for k in range(KC):
                nc.sync.dma_start(dbg["normx2"][k * P:(k + 1) * P, :], normx2[k][:])

        # ---- MLP ----
        h_stage = ctx.enter_context(ExitStack())
        h_pool = h_stage.enter_context(
            tc.tile_pool(name="h_pool", bufs=MFC1, side="right"))
        hT = []
        with tc.tile_pool(name="wf1_pool", bufs=KC) as wf1_pool:
            wf1 = []
            for k in range(KC):
                t = wf1_pool.tile([P, HID], BF16, tag="wfc1")
                nc.sync.dma_start(t[:], wfc1T[k * P:(k + 1) * P, :])
                wf1.append(t)
            for m in range(MFC1):
                ps = psmm.tile([P, NT], F32, tag="mm")
                for hs in _halves():
                    for k in range(KC):
                        nc.tensor.matmul(ps[:, hs],
                                         wf1[k][:, m * P:(m + 1) * P],
                                         normx2[k][:, hs],
                                         start=(k == 0), stop=(k == KC - 1))
                ht = h_pool.tile([P, NT], BF16, tag="h")
                nc.scalar.activation(ht[:], ps[:], AF.Gelu,
                                     bias=f1b[:, m:m + 1])
                hT.append(ht)
                if DEBUG:
                    nc.sync.dma_start(dbg["hT"][m * P:(m + 1) * P, :], ht[:])
        nx2_stage.close()

        with tc.tile_pool(name="wf2_pool", bufs=MFC1) as wf2_pool, \
             tc.tile_pool(name="o_pool", bufs=3) as o_pool, \
             tc.tile_pool(name="f2_pool", bufs=2) as f2_pool:
            wf2 = []
            for k in range(MFC1):
                t = wf2_pool.tile([P, D], BF16, tag="wfc2")
                nc.sync.dma_start(t[:], wfc2T[k * P:(k + 1) * P, :])
                wf2.append(t)
            for m in range(KC):
                ps = psmm.tile([P, NT], F32, tag="mm")
                for hs in _halves():
                    for k in range(MFC1):
                        nc.tensor.matmul(ps[:, hs],
                                         wf2[k][:, m * P:(m + 1) * P],
                                         hT[k][:, hs],
                                         start=(k == 0), stop=(k == MFC1 - 1))
                tm = f2_pool.tile([P, NT], F32, tag="tm")
                nc.scalar.activation(tm[:], ps[:], AF.Identity,
                                     bias=f2b[:, m:m + 1])
                ok = o_pool.tile([P, NT], F32, tag="o")
                nc.vector.tensor_add(ok[:], x1[m][:], tm[:])
                for qr in range(2):
                    nc.sync.dma_start(
                        outT[m * P + qr * HD:m * P + (qr + 1) * HD, :],
                        ok[qr * HD:(qr + 1) * HD, :])
        h_stage.close()

    nc.compile()
    return nc


_NC = None


def _get_nc():
    global _NC
    if _NC is None:
        _NC = build()
    return _NC


def _prep_inputs(inputs):
    f32 = np.float32
    bf16 = ml_dtypes.bfloat16
    g = {k: np.asarray(v) for k, v in inputs.items()}
    shared = {
        "wqkvT": np.ascontiguousarray(g["qkv_w"].astype(f32).T).astype(bf16),
        "wprojT": np.ascontiguousarray(g["proj_w"].astype(f32).T).astype(bf16),
        "wfc1T": np.ascontiguousarray(g["fc1_w"].astype(f32).T).astype(bf16),
        "wfc2T": np.ascontiguousarray(g["fc2_w"].astype(f32).T).astype(bf16),
        "pbias": np.ascontiguousarray(g["proj_b"], dtype=f32),
        "fc1b": np.ascontiguousarray(g["fc1_b"], dtype=f32),
        "fc2b": np.ascontiguousarray(g["fc2_b"], dtype=f32),
        "g1": np.concatenate([g["ln1a_g"], g["ln1b_g"], g["ln1c_g"]]).astype(f32),
        "b1": np.concatenate([g["ln1a_b"], g["ln1b_b"], g["ln1c_b"]]).astype(f32),
        "g2": np.concatenate([g["ln2a_g"], g["ln2b_g"], g["ln2c_g"]]).astype(f32),
        "b2": np.concatenate([g["ln2a_b"], g["ln2b_b"], g["ln2c_b"]]).astype(f32),
    }
    ind = np.zeros((D, 3), dtype=f32)
    ind[0:S1, 0] = 1.0
    ind[S1:S2, 1] = 1.0
    ind[S2:D, 2] = 1.0
    ind_pad = np.zeros((D, P), dtype=f32)
    ind_pad[:, 0:3] = ind
    shared["indsum"] = ind_pad.astype(bf16)
    shared["indbT"] = np.ascontiguousarray(ind.T)
    shared["invlen"] = np.array([[1.0 / S1], [1.0 / (S2 - S1)],
                                 [1.0 / (D - S2)]], dtype=f32)
    x = np.asarray(g["x"], dtype=f32)
    in_maps = []
    for b in range(B):
        m = dict(shared)
        m["xT"] = np.ascontiguousarray(x[b].T)
        in_maps.append(m)
    return in_maps


def run(inputs, trace=False):
    nc = _get_nc()
    in_maps = _prep_inputs(inputs)
    res = run_bass_kernel_spmd(nc, in_maps, core_ids=list(range(B)),
                               trace=trace)
    out = np.stack([np.ascontiguousarray(res.results[b]["outT"].T)
                    for b in range(B)]).astype(np.float32)
    return out, res


def kernel(**inputs):
    out, _ = run(inputs, trace=False)
    return out


# revision 25
# speedup vs baseline: 1.0284x; 1.0284x over previous
"""Trainium2 Bass kernel for a dense transformer block.

Block: split-LayerNorm -> attention -> residual -> split-LayerNorm -> MLP(GELU)
-> residual.  Shapes: B=8, N=1024, D=768, H=12 heads (hd=64), HID=3072.

Sharding: pure data-parallel over batch -- one batch element per NeuronCore
(8 cores), all weights replicated, no collectives.

On-chip layout is feature-major (activations stored transposed, [feature, token])
so every matmul consumes activations as the moving operand directly and
LayerNorm/softmax cross-feature reductions map onto PE matmuls with
indicator/ones vectors.  The host pre-transposes x and all weight matrices so
every DMA is contiguous.
"""

import os
import numpy as np
import ml_dtypes

import concourse.bass as bass
import concourse.tile as tile
from concourse import bacc, mybir
from concourse.bass_utils import run_bass_kernel_spmd
from concourse.masks import make_identity
from contextlib import ExitStack

F32 = mybir.dt.float32
BF16 = mybir.dt.bfloat16
F32R = mybir.dt.float32r
AF = mybir.ActivationFunctionType
ALU = mybir.AluOpType

D = 768
H = 12
HD = 64
HID = 3072
NT = 1024  # tokens per core
B = 8
S1 = 320  # split-LN segment boundaries: [0,320), [320,384), [384,768)
S2 = 384
SCALE = 0.125  # (D//H) ** -0.5 = 64 ** -0.5
EPS = 1e-5
P = 128

KC = D // P        # 6  c-chunks
MQKV = 3 * D // P  # 18 qkv out chunks
MFC1 = HID // P    # 24


def _halves():
    return (slice(0, 512), slice(512, 1024))


# Per segment: (ind_bT column range for the broadcast lhsT,
#               [(chunk, row0, row1), ...] applied regions)
LN_REGIONS = [
    (slice(0, P), [(0, 0, P), (1, 0, P), (2, 0, HD)]),        # seg0 [0,320)
    (slice(S1, S2), [(2, HD, P)]),                            # seg1 [320,384)
    (slice(S2, S2 + P), [(3, 0, P), (4, 0, P), (5, 0, P)]),   # seg2 [384,768)
]


def _layernorm(tc, x_tiles, g_tile, b_tile, ind_sum_tiles, ind_bT, invlen,
               eps_t, psmm, out_pool, tag):
    """Split-LayerNorm over the feature dim (partitions).  x_tiles: 6 fp32
    [128,1024] feature-major tiles.  Returns 6 bf16 normalized tiles."""
    nc = tc.nc
    with tc.tile_pool(name=f"ln_{tag}", bufs=3) as lnp, \
         tc.tile_pool(name=f"lns_{tag}", bufs=1) as lns, \
         tc.tile_pool(name=f"lnps_{tag}", bufs=2, space="PSUM") as psstat:
        # segment sums via indicator matmuls: sums[s, q] = sum_{c in seg_s} x[c, q]
        sums_t = psstat.tile([P, NT], F32, tag="stat")
        sumsq_t = psstat.tile([P, NT], F32, tag="stat")
        sums = sums_t[0:3, :]
        sumsq = sumsq_t[0:3, :]
        for k in range(KC):
            xbk = lnp.tile([P, NT], BF16, tag="xb")
            nc.vector.tensor_copy(xbk[:], x_tiles[k][:])
            xqk = lnp.tile([P, NT], BF16, tag="xq")
            nc.vector.tensor_mul(xqk[:], x_tiles[k][:], x_tiles[k][:])
            for hs in _halves():
                nc.tensor.matmul(sums_t[:, hs], ind_sum_tiles[k][:],
                                 xbk[:, hs],
                                 start=(k == 0), stop=(k == KC - 1))
            for hs in _halves():
                nc.tensor.matmul(sumsq_t[:, hs], ind_sum_tiles[k][:],
                                 xqk[:, hs],
                                 start=(k == 0), stop=(k == KC - 1))
        # stats on [3, NT]
        mean = lns.tile([3, NT], F32, tag="mean")
        nc.vector.tensor_scalar_mul(mean[:], sums[:], invlen[:])
        nmsq = lns.tile([3, NT], F32, tag="nmsq")
        nc.vector.scalar_tensor_tensor(nmsq[:], mean[:], -1.0, mean[:],
                                       ALU.mult, ALU.mult)
        var = lns.tile([3, NT], F32, tag="var")
        nc.vector.scalar_tensor_tensor(var[:], sumsq[:], invlen[:], nmsq[:],
                                       ALU.mult, ALU.add)
        std = lns.tile([3, NT], F32, tag="std")
        nc.scalar.activation(std[:], var[:], AF.Sqrt, bias=eps_t[:])
        rstd = lns.tile([3, NT], F32, tag="rstd")
        scr = lns.tile([3, NT], F32, tag="scr")
        nc.vector.reciprocal_approx_accurate(rstd[:], std[:], scr[:])
        beta = lns.tile([3, NT], F32, tag="beta")
        nc.vector.scalar_tensor_tensor(beta[:], mean[:], -1.0, rstd[:],
                                       ALU.mult, ALU.mult)
        rstd_r = lns.tile([3, NT], F32R, tag="rstd_r")
        nc.vector.tensor_copy(rstd_r[:], rstd[:])
        beta_r = lns.tile([3, NT], F32R, tag="beta_r")
        nc.vector.tensor_copy(beta_r[:], beta[:])
        # Broadcast per-token stats across partitions (one f32r matmul per
        # segment, reused by every chunk-region in that segment) + apply.
        t1s = {}
        t2s = {}
        done_rows = {}
        out_tiles = [None] * KC
        for cols, regions in LN_REGIONS:
            m_rows = max(r1 - r0 for _, r0, r1 in regions)
            aB = psmm.tile([P, NT], F32, tag="mm")
            bB = psmm.tile([P, NT], F32, tag="mm")
            for hs in _halves():
                nc.tensor.matmul(aB[0:m_rows, hs],
                                 ind_bT[:, cols][:, 0:m_rows],
                                 rstd_r[:, hs], start=True, stop=True)
                nc.tensor.matmul(bB[0:m_rows, hs],
                                 ind_bT[:, cols][:, 0:m_rows],
                                 beta_r[:, hs], start=True, stop=True)
            for k, r0, r1 in regions:
                if k not in t1s:
                    t1k = lnp.tile([P, NT], F32, tag="t1")
                    t2k = lnp.tile([P, NT], F32, tag="t2")
                    t1s[k] = t1k
                    t2s[k] = t2k
                    done_rows[k] = 0
                gcol = g_tile[r0:r1, k:k + 1]
                # t1 = (x * g) * rstd_bcast ; t2 = (-mean*rstd*g) + t1
                nc.vector.scalar_tensor_tensor(
                    t1s[k][r0:r1, :], x_tiles[k][r0:r1, :], gcol,
                    aB[0:r1 - r0, :], ALU.mult, ALU.mult)
                nc.vector.scalar_tensor_tensor(
                    t2s[k][r0:r1, :], bB[0:r1 - r0, :], gcol,
                    t1s[k][r0:r1, :], ALU.mult, ALU.add)
                done_rows[k] += r1 - r0
                if done_rows[k] == P:
                    ok = out_pool.tile([P, NT], BF16, tag=f"normx_{tag}")
                    nc.scalar.activation(ok[:], t2s[k][:], AF.Identity,
                                         bias=b_tile[:, k:k + 1])
                    out_tiles[k] = ok
        return out_tiles


DEBUG = bool(int(os.environ.get("KBG_DEBUG", "0")))


def build():
    nc = bacc.Bacc("TRN2", target_bir_lowering=False, debug=False)
    dbg = {}
    if DEBUG:
        dbg["normx"] = nc.dram_tensor("dbg_normx", [D, NT], BF16, kind="ExternalOutput")
        dbg["qkvT"] = nc.dram_tensor("dbg_qkvT", [3 * D, NT], BF16, kind="ExternalOutput")
        dbg["yt"] = nc.dram_tensor("dbg_yt", [D, NT], BF16, kind="ExternalOutput")
        dbg["x1"] = nc.dram_tensor("dbg_x1", [D, NT], F32, kind="ExternalOutput")
        dbg["normx2"] = nc.dram_tensor("dbg_normx2", [D, NT], BF16, kind="ExternalOutput")
        dbg["hT"] = nc.dram_tensor("dbg_hT", [HID, NT], BF16, kind="ExternalOutput")


    xT = nc.dram_tensor("xT", [D, NT], F32, kind="ExternalInput")
    wqkvT = nc.dram_tensor("wqkvT", [D, 3 * D], BF16, kind="ExternalInput")
    wprojT = nc.dram_tensor("wprojT", [D, D], BF16, kind="ExternalInput")
    wfc1T = nc.dram_tensor("wfc1T", [D, HID], BF16, kind="ExternalInput")
    wfc2T = nc.dram_tensor("wfc2T", [HID, D], BF16, kind="ExternalInput")
    pbias = nc.dram_tensor("pbias", [D], F32, kind="ExternalInput")
    fc1b = nc.dram_tensor("fc1b", [HID], F32, kind="ExternalInput")
    fc2b = nc.dram_tensor("fc2b", [D], F32, kind="ExternalInput")
    g1d = nc.dram_tensor("g1", [D], F32, kind="ExternalInput")
    b1d = nc.dram_tensor("b1", [D], F32, kind="ExternalInput")
    g2d = nc.dram_tensor("g2", [D], F32, kind="ExternalInput")
    b2d = nc.dram_tensor("b2", [D], F32, kind="ExternalInput")
    indsum = nc.dram_tensor("indsum", [D, P], BF16, kind="ExternalInput")
    indbTd = nc.dram_tensor("indbT", [3, D], F32R, kind="ExternalInput")
    invlend = nc.dram_tensor("invlen", [3, 1], F32, kind="ExternalInput")
    outT = nc.dram_tensor("outT", [D, NT], F32, kind="ExternalOutput")

    with tile.TileContext(nc) as tc, ExitStack() as ctx:
        const = ctx.enter_context(tc.tile_pool(name="const", bufs=1))
        psmm = ctx.enter_context(tc.tile_pool(name="psmm", bufs=2, space="PSUM"))

        # constants
        eps_t = const.tile([3, 1], F32)
        nc.vector.memset(eps_t[:], EPS)
        ones64 = const.tile([1, HD], F32)
        nc.vector.memset(ones64[:], 1.0)

        def load_cols(dram, n):
            t = const.tile([P, n], F32, tag=f"c_{dram.name}")
            nc.sync.dma_start(t[:], dram.ap().rearrange("(a p) -> p a", p=P))
            return t

        pb = load_cols(pbias, KC)
        f1b = load_cols(fc1b, MFC1)
        f2b = load_cols(fc2b, KC)
        g1 = load_cols(g1d, KC)
        b1 = load_cols(b1d, KC)
        g2 = load_cols(g2d, KC)
        b2 = load_cols(b2d, KC)
        ind_sum_tiles = []
        for k in range(KC):
            t = const.tile([P, P], BF16, tag=f"inds{k}")
            nc.sync.dma_start(t[:], indsum[k * P:(k + 1) * P, :])
            ind_sum_tiles.append(t)
        ind_bT = const.tile([3, D], F32R)
        nc.sync.dma_start(ind_bT[:], indbTd[:])
        invlen = const.tile([3, 1], F32)
        nc.sync.dma_start(invlen[:], invlend[:])

        x1pool = ctx.enter_context(tc.tile_pool(name="x1pool", bufs=KC))

        stage1 = ctx.enter_context(ExitStack())  # x0, lives through proj
        x0pool = stage1.enter_context(tc.tile_pool(name="x0pool", bufs=KC))
        x_tiles = []
        for k in range(KC):
            t = x0pool.tile([P, NT], F32, tag="x0")
            for qr in range(4):
                nc.sync.dma_start(t[qr * 32:(qr + 1) * 32, :],
                                  xT[k * P + qr * 32:k * P + (qr + 1) * 32, :])
            x_tiles.append(t)

        # ---- LN1 ----
        nx_stage = ctx.enter_context(ExitStack())  # normx, lives through qkv
        nx_pool = nx_stage.enter_context(tc.tile_pool(name="nx_pool", bufs=KC))
        normx = _layernorm(tc, x_tiles, g1, b1, ind_sum_tiles, ind_bT,
                           invlen, eps_t, psmm, nx_pool, "ln1")
        if DEBUG:
            for k in range(KC):
                nc.sync.dma_start(dbg["normx"][k * P:(k + 1) * P, :], normx[k][:])

        # ---- fused QKV + attention, one head-pair at a time ----
        # Pair j computes qkv chunks {j, 6+j, 12+j}, builds the pair's V
        # tiles, then runs both heads.  Attention's ACT-heavy exp stream
        # overlaps the next pair's PE-heavy qkv matmuls.
        y_stage = ctx.enter_context(ExitStack())
        y_pool = y_stage.enter_context(
            tc.tile_pool(name="y_pool", bufs=KC, side="right"))
        qkv_stage = ctx.enter_context(ExitStack())
        q_pool = qkv_stage.enter_context(
            tc.tile_pool(name="q_pool", bufs=9, side="right"))
        yt = []
        for _yi in range(KC):
            yt_t = y_pool.tile([P, NT], BF16, tag="yt")
            yt.append(yt_t)
        with tc.tile_pool(name="wqkv", bufs=KC) as wq_pool, \
             tc.tile_pool(name="v_pool", bufs=4) as v_pool, \
             tc.tile_pool(name="e_pool", bufs=3) as e_pool, \
             tc.tile_pool(name="kp_pool", bufs=3) as kp_pool, \
             tc.tile_pool(name="psot", bufs=2, space="PSUM") as psot, \
             tc.tile_pool(name="sm_pool", bufs=2) as sm_pool:
            wq = []
            for k in range(KC):
                t = wq_pool.tile([P, 3 * D], BF16, tag="wqkv")
                nc.sync.dma_start(t[:], wqkvT[k * P:(k + 1) * P, :])
                wq.append(t)

            # Per-head normalization tail, deferred into the next head's
            # loop so the DVE reciprocal latency hides under its matmuls.
            def make_tail(h, ot):
                po = (h % 2) * HD

                def tail():
                    dnm = sm_pool.tile([1, NT], F32, tag="dnm")
                    nc.vector.tensor_copy(dnm[:], ot[64:65, :])
                    r = sm_pool.tile([1, NT], F32, tag="recip")
                    nc.vector.reciprocal_approx_fast(r[:], dnm[:])
                    rbs = sm_pool.tile([HD, NT], F32, tag="rbs")
                    nc.gpsimd.partition_broadcast(rbs[:], r[:])
                    nc.vector.tensor_mul(yt[h // 2][po:po + HD, :],
                                         ot[0:HD, :], rbs[:])
                return tail

            pending_tail = None
            for j in range(KC):
                qkvT_j = {}
                for m in (j, 6 + j, 12 + j):
                    ps = psmm.tile([P, NT], F32, tag="mm")
                    for hs in _halves():
                        for k in range(KC):
                            nc.tensor.matmul(ps[:, hs],
                                             wq[k][:, m * P:(m + 1) * P],
                                             normx[k][:, hs],
                                             start=(k == 0), stop=(k == KC - 1))
                    qt = q_pool.tile([P, NT], BF16, tag="qkv")
                    nc.scalar.activation(qt[:], ps[:], AF.Copy)
                    qkvT_j[m] = qt
                    if DEBUG:
                        nc.sync.dma_start(dbg["qkvT"][m * P:(m + 1) * P, :],
                                          qt[:])
                # V token-major via DMA transpose (plus the all-ones
                # columns from the memset giving the softmax denominator at
                # output row 64)
                vts = []
                vsl = qkvT_j[12 + j]
                for hh in range(2):
                    po = hh * HD
                    vt = v_pool.tile([P, 8 * P], BF16, tag="vaug")
                    nc.vector.memset(vt[:], 1.0)
                    for kc in range(8):
                        nc.sync.dma_start(
                            vt[:, kc * P:kc * P + HD],
                            vsl[po:po + HD, kc * P:(kc + 1) * P],
                            transpose=True)
                    vts.append(vt)
                for hh in range(2):
                    h = 2 * j + hh
                    po = hh * HD
                    qsl = qkvT_j[j]
                    ksl = qkvT_j[6 + j]
                    # K=128 zero-padded k tile: rows [po:po+64] hold this
                    # head's k, the other 64 rows are zero so the full-height
                    # q rhs contributes nothing outside this head (and the
                    # weight load takes the fast NumWeights==128 path).
                    kp = kp_pool.tile([P, NT], BF16, tag="kp")
                    nc.vector.memset(kp[HD - po:P - po, :], 0.0)
                    nc.vector.tensor_copy(kp[po:po + HD, :],
                                          ksl[po:po + HD, :])
                    ot = psot.tile([P, NT], F32, tag="ot")
                    exs = []
                    for kc in range(8):
                        st = psmm.tile([P, NT], F32, tag="mm")
                        for hs in _halves():
                            nc.tensor.matmul(
                                st[:, hs],
                                kp[:, kc * P:(kc + 1) * P],
                                qsl[:, hs],
                                start=True, stop=True)
                        ex = e_pool.tile([P, NT], BF16, tag="expst")
                        nc.scalar.activation(ex[:], st[:], AF.Exp, scale=SCALE)
                        exs.append(ex)
                        if kc >= 1:
                            exp_prev = exs[kc - 1]
                            for hs in _halves():
                                nc.tensor.matmul(
                                    ot[:, hs],
                                    vts[hh][:, (kc - 1) * P:(kc - 1) * P + P],
                                    exp_prev[:, hs],
                                    start=(kc == 1), stop=False)
                        if kc == 3 and pending_tail is not None:
                            pending_tail()
                            pending_tail = None
                    for hs in _halves():
                        nc.tensor.matmul(ot[:, hs],
                                         vts[hh][:, 7 * P:7 * P + P],
                                         exs[7][:, hs],
                                         start=False, stop=True)
                    pending_tail = make_tail(h, ot)
            pending_tail()
            if DEBUG:
                for k in range(KC):
                    nc.sync.dma_start(dbg["yt"][k * P:(k + 1) * P, :], yt[k][:])
        nx_stage.close()
        qkv_stage.close()

        # ---- proj + residual ----
        x1 = []
        with tc.tile_pool(name="wp_pool", bufs=KC) as wp_pool, \
             tc.tile_pool(name="prj_pool", bufs=2) as prj_pool:
            wp = []
            for k in range(KC):
                t = wp_pool.tile([P, D], BF16, tag="wp")
                nc.sync.dma_start(t[:], wprojT[k * P:(k + 1) * P, :])
                wp.append(t)
            for m in range(KC):
                ps = psmm.tile([P, NT], F32, tag="mm")
                for hs in _halves():
                    for k in range(KC):
                        nc.tensor.matmul(ps[:, hs],
                                         wp[k][:, m * P:(m + 1) * P],
                                         yt[k][:, hs],
                                         start=(k == 0), stop=(k == KC - 1))
                tp = prj_pool.tile([P, NT], F32, tag="tp")
                nc.scalar.activation(tp[:], ps[:], AF.Identity,
                                     bias=pb[:, m:m + 1])
                xk = x1pool.tile([P, NT], F32, tag="x1")
                nc.vector.tensor_add(xk[:], x_tiles[m][:], tp[:])
                x1.append(xk)
                if DEBUG:
                    nc.sync.dma_start(dbg["x1"][m * P:(m + 1) * P, :], xk[:])
        y_stage.close()
        stage1.close()

        # ---- LN2 ----
        nx2_stage = ctx.enter_context(ExitStack())
        nx2_pool = nx2_stage.enter_context(tc.tile_pool(name="nx2_pool",
                                                        bufs=KC))
        normx2 = _layernorm(tc, x1, g2, b2, ind_sum_tiles, ind_bT,
                            invlen, eps_t, psmm, nx2_pool, "ln2")
        if DEBUG:
            for k in range(KC):
                nc.sync.dma_start(dbg["normx2"][k * P:(k + 1) * P, :], normx2[k][:])

        # ---- MLP ----
        h_stage = ctx.enter_context(ExitStack())
        h_pool = h_stage.enter_context(
            tc.tile_pool(name="h_pool", bufs=MFC1, side="right"))
        hT = []
        with tc.tile_pool(name="wf1_pool", bufs=KC) as wf1_pool:
            wf1 = []
            for k in range(KC):
                t = wf1_pool.tile([P, HID], BF16, tag="wfc1")
                nc.sync.dma_start(t[:], wfc1T[k * P:(k + 1) * P, :])
                wf1.append(t)
            for m in range(MFC1):
                ps = psmm.tile([P, NT], F32, tag="mm")
                for hs in _halves():
                    for k in range(KC):
                        nc.tensor.matmul(ps[:, hs],
                                         wf1[k][:, m * P:(m + 1) * P],
                                         normx2[k][:, hs],
                                         start=(k == 0), stop=(k == KC - 1))
                ht = h_pool.tile([P, NT], BF16, tag="h")
                nc.scalar.activation(ht[:], ps[:], AF.Gelu,
                                     bias=f1b[:, m:m + 1])
                hT.append(ht)
                if DEBUG:
                    nc.sync.dma_start(dbg["hT"][m * P:(m + 1) * P, :], ht[:])
        nx2_stage.close()

        with tc.tile_pool(name="wf2_pool", bufs=MFC1) as wf2_pool, \
             tc.tile_pool(name="o_pool", bufs=3) as o_pool, \
             tc.tile_pool(name="f2_pool", bufs=2) as f2_pool:
            wf2 = []
            for k in range(MFC1):
                t = wf2_pool.tile([P, D], BF16, tag="wfc2")
                nc.sync.dma_start(t[:], wfc2T[k * P:(k + 1) * P, :])
                wf2.append(t)
            for m in range(KC):
                ps = psmm.tile([P, NT], F32, tag="mm")
                for hs in _halves():
                    for k in range(MFC1):
                        nc.tensor.matmul(ps[:, hs],
                                         wf2[k][:, m * P:(m + 1) * P],
                                         hT[k][:, hs],
                                         start=(k == 0), stop=(k == MFC1 - 1))
                tm = f2_pool.tile([P, NT], F32, tag="tm")
                nc.scalar.activation(tm[:], ps[:], AF.Identity,
                                     bias=f2b[:, m:m + 1])
                ok = o_pool.tile([P, NT], F32, tag="o")
                nc.vector.tensor_add(ok[:], x1[m][:], tm[:])
                for qr in range(2):
                    nc.sync.dma_start(
                        outT[m * P + qr * HD:m * P + (qr + 1) * HD, :],
                        ok[qr * HD:(qr + 1) * HD, :])
        h_stage.close()

    nc.compile()
    return nc


_NC = None


def _get_nc():
    global _NC
    if _NC is None:
        _NC = build()
    return _NC


def _prep_inputs(inputs):
    f32 = np.float32
    bf16 = ml_dtypes.bfloat16
    g = {k: np.asarray(v) for k, v in inputs.items()}
    shared = {
        "wqkvT": np.ascontiguousarray(g["qkv_w"].astype(f32).T).astype(bf16),
        "wprojT": np.ascontiguousarray(g["proj_w"].astype(f32).T).astype(bf16),
        "wfc1T": np.ascontiguousarray(g["fc1_w"].astype(f32).T).astype(bf16),
        "wfc2T": np.ascontiguousarray(g["fc2_w"].astype(f32).T).astype(bf16),
        "pbias": np.ascontiguousarray(g["proj_b"], dtype=f32),
        "fc1b": np.ascontiguousarray(g["fc1_b"], dtype=f32),
        "fc2b": np.ascontiguousarray(g["fc2_b"], dtype=f32),
        "g1": np.concatenate([g["ln1a_g"], g["ln1b_g"], g["ln1c_g"]]).astype(f32),
        "b1": np.concatenate([g["ln1a_b"], g["ln1b_b"], g["ln1c_b"]]).astype(f32),
        "g2": np.concatenate([g["ln2a_g"], g["ln2b_g"], g["ln2c_g"]]).astype(f32),
        "b2": np.concatenate([g["ln2a_b"], g["ln2b_b"], g["ln2c_b"]]).astype(f32),
    }
    ind = np.zeros((D, 3), dtype=f32)
    ind[0:S1, 0] = 1.0
    ind[S1:S2, 1] = 1.0
    ind[S2:D, 2] = 1.0
    ind_pad = np.zeros((D, P), dtype=f32)
    ind_pad[:, 0:3] = ind
    shared["indsum"] = ind_pad.astype(bf16)
    shared["indbT"] = np.ascontiguousarray(ind.T)
    shared["invlen"] = np.array([[1.0 / S1], [1.0 / (S2 - S1)],
                                 [1.0 / (D - S2)]], dtype=f32)
    x = np.asarray(g["x"], dtype=f32)
    in_maps = []
    for b in range(B):
        m = dict(shared)
        m["xT"] = np.ascontiguousarray(x[b].T)
        in_maps.append(m)
    return in_maps


def run(inputs, trace=False):
    nc = _get_nc()
    in_maps = _prep_inputs(inputs)
    res = run_bass_kernel_spmd(nc, in_maps, core_ids=list(range(B)),
                               trace=trace)
    out = np.stack([np.ascontiguousarray(res.results[b]["outT"].T)
                    for b in range(B)]).astype(np.float32)
    return out, res


def kernel(**inputs):
    out, _ = run(inputs, trace=False)
    return out
